# revision 8
# baseline (speedup 1.0000x reference)
"""Trainium2 Bass kernel for the CAB fusion:

    out = shallower * sigmoid(MLP(concat(gap(shallower), gap(deeper)))) +
          bilinear_upsample_2x(deeper)

Sharding: pure data parallel - batch 16 split 2-per-core across 8
NeuronCores; tiny 1x1-conv weights replicated.

V2 schedule (bf16 end-to-end; tolerance gate is 2e-2 so bf16's ~0.4%
per-op rounding is cheap insurance against the fp32 version's
triple-bound profile):
  - All HBM traffic in bf16: 9.9 MB/core instead of 19.7 MB -> DMA
    floor ~26us at the ~390 GB/s/core measured aggregate.
  - DVE: W-pass (stride-2 interleave, 1 elem/cyc) then H-pass + finals
    (unit-stride bf16 -> 2x mode, 2 elem/cyc). Emission order
    W0,H0,W1,finals0,H1,finals1 keeps DVE streaming without waiting on
    the MLP's sigmoid.
  - Pools are whole-tile Copy+accum on ScalarE (shallow tiles in place,
    deeper from the packed dd tile into a scratch), so MLP layer 1 is 4
    accumulating bf16 matmuls per output block (24 LDW+MM pairs total
    vs 192 fp32r slices in V1, which made PE 97% busy).
  - deeper arrives pre-scaled by 1/16 so both separable 2x-bilinear
    passes are exact `3*a + b` ops; pool mean folds go into host-side
    w1 column scales (1/64 deeper since dd=d/16, 1/4096 shallow).
  - Loads split across the two HWDGE rings (sync + scalar) in
    consumption order; stores alternate rings per row-chunk with the
    last tile tapered so the post-compute DMA tail is short.

Numerics: bf16 storage/ALU, fp32 accumulation (PSUM + accum_out).
"""

import numpy as np
import ml_dtypes
from contextlib import ExitStack

import concourse.bacc as bacc
import concourse.tile as tile
import concourse.mybir as mybir
from concourse import bass_utils

F32 = mybir.dt.float32
BF16 = mybir.dt.bfloat16
AF = mybir.ActivationFunctionType
OP = mybir.AluOpType

N_CORES = 8
B, C = 16, 256
HD, WD = 32, 32
HS, WS = 64, 64
BL = B // N_CORES          # batches per core
G = C // 128               # channel groups of 128
DHW = HD * WD              # 1024


def _emit(ctx, tc, dd, sh, wpack, bpack, out):
    nc = tc.nc

    wpool = ctx.enter_context(tc.tile_pool(name="weights", bufs=1))
    stat = ctx.enter_context(tc.tile_pool(name="stat", bufs=1))
    sres = ctx.enter_context(tc.tile_pool(name="sres", bufs=1))
    up = ctx.enter_context(tc.tile_pool(name="up", bufs=2))
    ures = ctx.enter_context(tc.tile_pool(name="ures", bufs=1))
    psum = ctx.enter_context(tc.tile_pool(name="psum", bufs=1, space="PSUM"))

    # ---- loads, consumption order, split across the two HWDGE rings ----
    dd_sb = wpool.tile([128, BL, G, HD, WD], BF16, name="dd_sb")
    dd_ap = dd.rearrange("p (b g x) -> p b g x", b=BL, g=G)
    dd_fl = dd_sb.rearrange("p b g h w -> p b g (h w)")
    s_sb = [sres.tile([128, G, HS, WS], BF16, name=f"s{b}")
            for b in range(BL)]
    wp_sb = wpool.tile([128, 6 * C], BF16, name="wp_sb")
    bp_sb = wpool.tile([128, 4], F32, name="bp_sb")

    # sync ring: dd00, s00, dd10, s10 / scalar: dd01, wp, s01, dd11, s11, bp
    nc.sync.dma_start(dd_fl[:, 0, 0], dd_ap[:, 0, 0])
    nc.scalar.dma_start(dd_fl[:, 0, 1], dd_ap[:, 0, 1])
    nc.scalar.dma_start(wp_sb[:], wpack[:, :])
    nc.sync.dma_start(s_sb[0][:, 0], sh[0, 0:128])
    nc.scalar.dma_start(s_sb[0][:, 1], sh[0, 128:256])
    nc.sync.dma_start(dd_fl[:, 1, 0], dd_ap[:, 1, 0])
    nc.scalar.dma_start(dd_fl[:, 1, 1], dd_ap[:, 1, 1])
    nc.sync.dma_start(s_sb[1][:, 0], sh[1, 0:128])
    nc.scalar.dma_start(s_sb[1][:, 1], sh[1, 128:256])
    nc.scalar.dma_start(bp_sb[:], bpack[:, :])
    wmat = wp_sb.rearrange("p (k o) -> p k o", k=6)

    # Preload the sigmoid LUT off the critical path (dummy op on zeros),
    # emitted after all load triggers.
    sig_warm = stat.tile([128, 1], F32, name="sig_warm")
    nc.gpsimd.memset(sig_warm[:], 0.0)
    nc.scalar.activation(sig_warm[:], sig_warm[:], AF.Sigmoid)

    # ---- pools + MLP state ----
    cols_f = [stat.tile([128, 4], F32, name=f"colsf{b}") for b in range(BL)]
    cols_b = [stat.tile([128, 4], BF16, name=f"colsb{b}") for b in range(BL)]
    scr = stat.tile([128, HD, WD], BF16, name="scr")
    sig_t = stat.tile([128, G, BL], F32, name="sig_t")
    ht = [stat.tile([128, BL], BF16, name=f"h{og}") for og in range(G)]

    def pools(b):
        # cols order: [shallow g0, shallow g1, deeper g0, deeper g1]
        for g in range(G):
            nc.scalar.activation(scr[:], dd_sb[:, b, g], AF.Copy,
                                 accum_out=cols_f[b][:, 2 + g:3 + g])
        for g in range(G):
            nc.scalar.activation(s_sb[b][:, g], s_sb[b][:, g], AF.Copy,
                                 accum_out=cols_f[b][:, g:g + 1])
        nc.scalar.activation(cols_b[b][:], cols_f[b][:], AF.Copy)

    def mlp(b):
        bb = slice(b, b + 1)
        # layer 1: deeper chunks first (their pools are ready earliest)
        for og in range(G):
            ph = psum.tile([128, 1], F32, name=f"ph{og}{b}")
            ogs = slice(og * 128, (og + 1) * 128)
            chunks = [2, 3, 0, 1]
            for i, ck in enumerate(chunks):
                nc.tensor.matmul(ph[:], wmat[:, ck, ogs],
                                 cols_b[b][:, ck:ck + 1],
                                 start=(i == 0), stop=(i == len(chunks) - 1))
            nc.scalar.activation(ht[og][:, bb], ph[:], AF.Relu,
                                 bias=bp_sb[:, og:og + 1])
        for g in range(G):
            pg = psum.tile([128, 1], F32, name=f"pg{g}{b}")
            gs_ = slice(g * 128, (g + 1) * 128)
            for ig in range(G):
                nc.tensor.matmul(pg[:], wmat[:, 4 + ig, gs_], ht[ig][:, bb],
                                 start=(ig == 0), stop=(ig == 1))
            nc.scalar.activation(sig_t[:, g, bb], pg[:], AF.Sigmoid,
                                 bias=bp_sb[:, 2 + g:3 + g])

    # ---- upsample (DVE). STT has no DVE fast mode (1 elem/cyc always),
    # tensor_scalar reaches 4x and tensor_tensor 2x with unit-stride bf16.
    # So: W pass stays STT (mult+add in one 1x pass over the small tensor;
    # the stride-2 interleave forbids fast modes anyway), H pass becomes
    # zp3 = 3*yp (TS, 4x) + phase adds (TT, 2x), and the finals become
    # gate (TS, 4x, in place) + add (TT, 2x, in place).
    u_sb = []

    def upsample_w(b):
        yp = up.tile([128, G, HD, WS], BF16, name="yp")
        ypv = yp.rearrange("p g h (j t) -> p g h j t", t=2)
        for g in range(G):
            d = dd_sb[:, b, g]
            nc.vector.scalar_tensor_tensor(
                ypv[:, g, :, 1:WD, 0], d[:, :, 1:WD], 3.0, d[:, :, 0:WD - 1],
                OP.mult, OP.add)
            nc.vector.scalar_tensor_tensor(
                ypv[:, g, :, 0:WD - 1, 1], d[:, :, 0:WD - 1], 3.0,
                d[:, :, 1:WD], OP.mult, OP.add)
        ye = yp.rearrange("p g h w -> p (g h) w")
        dgh = dd_sb[:, b].rearrange("p g h w -> p (g h) w")
        nc.vector.tensor_scalar(ye[:, :, 0:WS:WS - 1],
                                dgh[:, :, 0:WD:WD - 1], 4.0, None, OP.mult)
        return yp

    def upsample_h(b, yp):
        u = ures.tile([128, G, HS, WS], BF16, name=f"u{b}")
        zp = up.tile([128, G, HD, WS], BF16, name="zp")
        nc.vector.tensor_scalar(
            zp.rearrange("p g h w -> p (g h w)"),
            yp.rearrange("p g h w -> p (g h w)"), 3.0, None, OP.mult)
        uv = u.rearrange("p g (i t) w -> p g i t w", t=2)
        for g in range(G):
            nc.vector.tensor_tensor(
                uv[:, g, 1:HD, 0, :], zp[:, g, 1:HD, :],
                yp[:, g, 0:HD - 1, :], OP.add)
            nc.vector.tensor_tensor(
                uv[:, g, 0:HD - 1, 1, :], zp[:, g, 0:HD - 1, :],
                yp[:, g, 1:HD, :], OP.add)
            nc.vector.tensor_scalar(u[:, g, 0:HS:HS - 1, :],
                                    yp[:, g, 0:HD:HD - 1, :], 4.0, None,
                                    OP.mult)
        u_sb.append(u)

    def finals(b):
        s = s_sb[b]
        u = u_sb[b]
        sfl = s.rearrange("p g h w -> p g (h w)")
        for g in range(G):
            nc.vector.tensor_scalar(sfl[:, g], sfl[:, g],
                                    sig_t[:, g, b:b + 1], None, OP.mult)
        # b0 in halves; b1 in quarters with the very last tile tapered so
        # the trailing store after the last VectorE op is short.
        for g in range(G):
            gs = slice(g * 128, (g + 1) * 128)
            if b == 0:
                bounds = [0, 32, 64]
            elif g == 0:
                bounds = [0, 16, 32, 48, 64]
            else:
                bounds = [0, 16, 32, 48, 56, 64]
            for q in range(len(bounds) - 1):
                rows = slice(bounds[q], bounds[q + 1])
                eng = nc.sync if (g + q) % 2 == 0 else nc.scalar
                nc.vector.tensor_tensor(
                    s[:, g, rows, :], s[:, g, rows, :], u[:, g, rows, :],
                    OP.add)
                eng.dma_start(out[b, gs, rows, :], s[:, g, rows, :])

    # ---- schedule: DVE stream is W0, zp0+H0, W1, gate0+finals0, zp1+H1,
    # finals1 -- finals(0) sits after W1 so the sigmoid has extra slack
    # before DVE needs it.
    pools(0)
    yp0 = upsample_w(0)
    mlp(0)
    upsample_h(0, yp0)
    pools(1)
    yp1 = upsample_w(1)
    mlp(1)
    finals(0)
    upsample_h(1, yp1)
    finals(1)


def build_kernel():
    nc = bacc.Bacc("TRN2", target_bir_lowering=False, debug=False,
                   num_devices=N_CORES)
    dd = nc.dram_tensor("dd", [128, BL * G * DHW], BF16,
                        kind="ExternalInput").ap()
    sh = nc.dram_tensor("sh", [BL, C, HS, WS], BF16,
                        kind="ExternalInput").ap()
    wpack = nc.dram_tensor("wpack", [128, 6 * C], BF16,
                           kind="ExternalInput").ap()
    bpack = nc.dram_tensor("bpack", [128, 4], F32,
                           kind="ExternalInput").ap()
    out = nc.dram_tensor("out", [BL, C, HS, WS], BF16,
                         kind="ExternalOutput").ap()

    with tile.TileContext(nc) as tc, ExitStack() as ctx:
        _emit(ctx, tc, dd, sh, wpack, bpack, out)
    nc.compile()
    return nc


_NC = None


def _get_nc():
    global _NC
    if _NC is None:
        _NC = build_kernel()
    return _NC


def prepare_in_maps(deeper, shallower, w1, b1, w2, b2):
    bf = ml_dtypes.bfloat16
    # w1t transposed with per-chunk mean folds: shallow rows (0:256) carry
    # the 1/(64*64) shallow-pool mean; deeper rows (256:512) carry 1/64
    # because dd holds d/16 and the deeper mean is sum(d)/1024.
    w1t = np.ascontiguousarray(w1.T).astype(np.float32)     # [512, 256]
    w1t[0:256] *= np.float32(1.0 / 4096.0)
    w1t[256:512] *= np.float32(1.0 / 64.0)
    w2t = np.ascontiguousarray(w2.T).astype(np.float32)     # [256, 256]
    wp = np.empty((128, 6 * C), np.float32)
    # blocks: 0=shallow g0, 1=shallow g1, 2=deeper g0, 3=deeper g1
    wp[:, 0 * C:1 * C] = w1t[0:128]
    wp[:, 1 * C:2 * C] = w1t[128:256]
    wp[:, 2 * C:3 * C] = w1t[256:384]
    wp[:, 3 * C:4 * C] = w1t[384:512]
    wp[:, 4 * C:5 * C] = w2t[0:128]
    wp[:, 5 * C:6 * C] = w2t[128:256]
    wp = wp.astype(bf)
    bp = np.empty((128, 4), np.float32)
    bp[:, 0:2] = b1.astype(np.float32).reshape(2, 128).T
    bp[:, 2:4] = b2.astype(np.float32).reshape(2, 128).T
    d16 = (deeper.astype(np.float32) * np.float32(1.0 / 16.0)).astype(bf)
    sh = shallower.astype(np.float32).astype(bf)
    in_maps = []
    for i in range(N_CORES):
        # dd pack: [BL, G, 128, HW] -> [128, BL*G*HW] (partition-major)
        dc = d16[i * BL:(i + 1) * BL].reshape(BL, G, 128, DHW)
        ddp = np.ascontiguousarray(
            dc.transpose(2, 0, 1, 3).reshape(128, BL * G * DHW))
        in_maps.append({
            "dd": ddp, "wpack": wp, "bpack": bp,
            "sh": np.ascontiguousarray(sh[i * BL:(i + 1) * BL]),
        })
    return in_maps


def gather(results):
    return np.concatenate(
        [results[i]["out"] for i in range(N_CORES)], axis=0
    ).astype(np.float32)


def kernel(deeper, shallower, w1, b1, w2, b2):
    nc = _get_nc()
    in_maps = prepare_in_maps(deeper, shallower, w1, b1, w2, b2)
    res = bass_utils.run_bass_kernel_spmd(nc, in_maps, list(range(N_CORES)))
    return gather(res.results)


# revision 29
# speedup vs baseline: 1.2183x; 1.2183x over previous
"""Trainium2 Bass kernel for the CAB fusion:

    out = shallower * sigmoid(MLP(concat(gap(shallower), gap(deeper)))) +
          bilinear_upsample_2x(deeper)

Sharding: pure data parallel - batch 16 split 2-per-core across 8
NeuronCores; tiny 1x1-conv weights replicated.

V4 schedule. Everything bf16 (gate is 2e-2; bf16 costs ~6e-3), built
around three HW-measured DVE facts:
  - scalar_tensor_tensor has NO fast mode (1 elem/cyc); tensor_scalar
    reaches 4x and tensor_tensor 2x with unit-stride bf16 operands.
  - every DVE op pays ~370 cycles of fixed cost, so work is folded into
    the fewest, largest ops possible.
  - stride-2 (interleaved) writes forbid fast modes entirely.
Consequences:
  - host packs deeper with edge-replicated guard rows/cols (34x34) so
    both bilinear passes run full-range with no separate edge ops;
  - host stores shallower (and reads out) in even/odd column PLANES, so
    the W pass writes contiguous planes: zd=3*d (TS 4x) + two plane TTs
    (2x). The H pass is zp3=3*yp (TS 4x) + two row-interleaved TTs (2x)
    over the (g h)-flattened space - u is padded to 68 rows per group so
    the 2:1 row mapping stays uniform across the group boundary
    (68 = 2*34), group-boundary garbage lands in the pad rows;
  - finals are gate s*=sig (TS 4x, per group) + s+=u (TT 2x, whole
    group for b0, tapered for b1 so the trailing store is short).
  - pools are whole-tile Copy+accum on ScalarE (shallow in place,
    deeper over the guarded tile's 32x32 interior); MLP layer 1 is 4
    accumulating bf16 matmuls per output block.

Numerics: bf16 storage/ALU, fp32 accumulation (PSUM + accum_out).
"""

import numpy as np
import ml_dtypes
from contextlib import ExitStack

import concourse.bacc as bacc
import concourse.tile as tile
import concourse.mybir as mybir
from concourse import bass_utils

F32 = mybir.dt.float32
BF16 = mybir.dt.bfloat16
AF = mybir.ActivationFunctionType
OP = mybir.AluOpType

N_CORES = 8
B, C = 16, 256
HD, WD = 32, 32
HS, WS = 64, 64
BL = B // N_CORES          # batches per core
G = C // 128               # channel groups of 128
HG = HD + 2                # guarded deeper extent (34)
DG = HG * HG               # guarded deeper tile size (1156)
HP = HS + 4                # padded upsample rows per group (68 = 2*34)


def _emit(ctx, tc, dd, sh, wpack, bpack, out):
    nc = tc.nc

    wpool = ctx.enter_context(tc.tile_pool(name="weights", bufs=1))
    stat = ctx.enter_context(tc.tile_pool(name="stat", bufs=1))
    sres = ctx.enter_context(tc.tile_pool(name="sres", bufs=1))
    up = ctx.enter_context(tc.tile_pool(name="up", bufs=2))
    ures = ctx.enter_context(tc.tile_pool(name="ures", bufs=1))
    psum = ctx.enter_context(tc.tile_pool(name="psum", bufs=1, space="PSUM"))

    # ---- loads, consumption order, split across the two HWDGE rings ----
    dd_sb = wpool.tile([128, BL, G, HG, HG], BF16, name="dd_sb")
    dd_ap = dd.rearrange("p (b g x) -> p b g x", b=BL, g=G)
    dd_fl = dd_sb.rearrange("p b g h w -> p b g (h w)")
    s_sb = [sres.tile([128, G, HS, WS], BF16, name=f"s{b}")
            for b in range(BL)]
    wp_sb = wpool.tile([128, 6 * C], BF16, name="wp_sb")
    bp_sb = wpool.tile([128, 4], F32, name="bp_sb")

    # sync ring: dd00, s00, dd10, s10 / scalar: dd01, wp, s01, dd11, s11, bp
    nc.sync.dma_start(dd_fl[:, 0, 0], dd_ap[:, 0, 0])
    nc.scalar.dma_start(dd_fl[:, 0, 1], dd_ap[:, 0, 1])
    nc.scalar.dma_start(wp_sb[:], wpack[:, :])
    nc.sync.dma_start(s_sb[0][:, 0], sh[0, 0:128])
    nc.scalar.dma_start(s_sb[0][:, 1], sh[0, 128:256])
    nc.sync.dma_start(dd_fl[:, 1, 0], dd_ap[:, 1, 0])
    nc.scalar.dma_start(dd_fl[:, 1, 1], dd_ap[:, 1, 1])
    nc.sync.dma_start(s_sb[1][:, 0], sh[1, 0:128])
    nc.scalar.dma_start(s_sb[1][:, 1], sh[1, 128:256])
    nc.scalar.dma_start(bp_sb[:], bpack[:, :])
    wmat = wp_sb.rearrange("p (k o) -> p k o", k=6)

    # Preload the sigmoid LUT off the critical path (dummy op on zeros),
    # emitted after all load triggers.
    sig_warm = stat.tile([128, 1], F32, name="sig_warm")
    nc.gpsimd.memset(sig_warm[:], 0.0)
    nc.scalar.activation(sig_warm[:], sig_warm[:], AF.Sigmoid)

    # ---- pools + MLP state ----
    # cols order: [shallow g0, shallow g1, deeper g0, deeper g1];
    # all pools are ScalarE Copy+accum (shallow in place, deeper via a
    # scratch from the guarded tile's 32x32 interior).
    cols_f = [stat.tile([128, 4], F32, name=f"colsf{b}") for b in range(BL)]
    cols_b = [stat.tile([128, 4], BF16, name=f"colsb{b}") for b in range(BL)]
    scr = stat.tile([128, HD, WD], BF16, name="scr")
    sig_t = stat.tile([128, G, BL], F32, name="sig_t")
    ht = [stat.tile([128, BL], BF16, name=f"h{og}") for og in range(G)]

    def pools(b):
        for g in range(G):
            nc.scalar.activation(scr[:], dd_sb[:, b, g, 1:HD + 1, 1:WD + 1],
                                 AF.Copy,
                                 accum_out=cols_f[b][:, 2 + g:3 + g])
        for g in range(G):
            nc.scalar.activation(s_sb[b][:, g], s_sb[b][:, g], AF.Copy,
                                 accum_out=cols_f[b][:, g:g + 1])
        nc.scalar.activation(cols_b[b][:], cols_f[b][:], AF.Copy)

    def mlp(b):
        bb = slice(b, b + 1)
        # layer 1: deeper chunks first (their pools are ready earliest)
        for og in range(G):
            ph = psum.tile([128, 1], F32, name=f"ph{og}{b}")
            ogs = slice(og * 128, (og + 1) * 128)
            chunks = [2, 3, 0, 1]
            for i, ck in enumerate(chunks):
                nc.tensor.matmul(ph[:], wmat[:, ck, ogs],
                                 cols_b[b][:, ck:ck + 1],
                                 start=(i == 0), stop=(i == len(chunks) - 1))
            nc.scalar.activation(ht[og][:, bb], ph[:], AF.Relu,
                                 bias=bp_sb[:, og:og + 1])
        for g in range(G):
            pg = psum.tile([128, 1], F32, name=f"pg{g}{b}")
            gs_ = slice(g * 128, (g + 1) * 128)
            for ig in range(G):
                nc.tensor.matmul(pg[:], wmat[:, 4 + ig, gs_], ht[ig][:, bb],
                                 start=(ig == 0), stop=(ig == 1))
            nc.scalar.activation(sig_t[:, g, bb], pg[:], AF.Sigmoid,
                                 bias=bp_sb[:, 2 + g:3 + g])

    # ---- upsample (DVE) ----
    u_sb = []

    def upsample_w(b):
        # planar W pass: yp[:, :, 0:32]=even cols, 32:64=odd cols.
        yp = up.tile([128, G, HG, WS], BF16, name="yp")
        zd = up.tile([128, G, HG, HG], BF16, name="zd")
        nc.vector.tensor_scalar(
            zd.rearrange("p g h w -> p (g h w)"),
            dd_sb[:, b].rearrange("p g h w -> p (g h w)"), 3.0, None, OP.mult)
        dgf = dd_sb[:, b].rearrange("p g h w -> p (g h) w")
        zdf = zd.rearrange("p g h w -> p (g h) w")
        ypf = yp.rearrange("p g h w -> p (g h) w")
        nc.vector.tensor_tensor(ypf[:, :, 0:HD], zdf[:, :, 1:HD + 1],
                                dgf[:, :, 0:HD], OP.add)
        nc.vector.tensor_tensor(ypf[:, :, HD:WS], zdf[:, :, 1:HD + 1],
                                dgf[:, :, 2:HD + 2], OP.add)
        return yp

    def upsample_h(b, yp):
        # u padded to 68 rows/group; row map real->flat stays uniform
        # across groups (68 = 2*34), boundary garbage lands in pads.
        u = ures.tile([128, G, HP, WS], BF16, name=f"u{b}")
        zp = up.tile([128, G, HG, WS], BF16, name="zp")
        nc.vector.tensor_scalar(
            zp.rearrange("p g h w -> p (g h w)"),
            yp.rearrange("p g h w -> p (g h w)"), 3.0, None, OP.mult)
        uf = u.rearrange("p g h w -> p (g h) w")
        zpf = zp.rearrange("p g h w -> p (g h) w")
        ypf = yp.rearrange("p g h w -> p (g h) w")
        n0 = G * HG - 1                        # 67 even-phase rows
        nc.vector.tensor_tensor(uf[:, 0:2 * n0:2, :], zpf[:, 1:n0 + 1, :],
                                ypf[:, 0:n0, :], OP.add)
        n1 = G * HG - 2                        # 66 odd-phase rows
        nc.vector.tensor_tensor(uf[:, 1:2 * n1:2, :], zpf[:, 1:n1 + 1, :],
                                ypf[:, 2:n1 + 2, :], OP.add)
        u_sb.append(u)

    def finals(b):
        s = s_sb[b]
        u = u_sb[b]
        sfl = s.rearrange("p g h w -> p g (h w)")
        for g in range(G):
            nc.vector.tensor_scalar(sfl[:, g], sfl[:, g],
                                    sig_t[:, g, b:b + 1], None, OP.mult)
        # b1's stores drain after the last DVE op, so they are balanced
        # to exactly 1MB per HWDGE ring (b1 g0: one whole-group TT, two
        # half stores; g1 tapered so the trailing store is short).
        for g in range(G):
            gs = slice(g * 128, (g + 1) * 128)
            if b == 0:
                nc.vector.tensor_tensor(
                    s[:, g], s[:, g], u[:, g, 0:HS, :], OP.add)
                eng = nc.sync if g == 0 else nc.scalar
                eng.dma_start(out[b, gs], s[:, g])
            elif g == 0:
                nc.vector.tensor_tensor(
                    s[:, g], s[:, g], u[:, g, 0:HS, :], OP.add)
                nc.sync.dma_start(out[b, gs, 0:32, :], s[:, g, 0:32, :])
                nc.scalar.dma_start(out[b, gs, 32:HS, :], s[:, g, 32:HS, :])
            else:
                bounds = [0, 32, 56, HS]
                rings = [nc.scalar, nc.sync, nc.sync]
                for q in range(len(bounds) - 1):
                    rows = slice(bounds[q], bounds[q + 1])
                    nc.vector.tensor_tensor(
                        s[:, g, rows, :], s[:, g, rows, :], u[:, g, rows, :],
                        OP.add)
                    rings[q].dma_start(out[b, gs, rows, :], s[:, g, rows, :])

    # ---- schedule: DVE stream W0, zp0+H0, W1, gate0+finals0, zp1+H1,
    # finals1 -- finals(0) sits after W1 so the sigmoid has extra slack.
    pools(0)
    yp0 = upsample_w(0)
    mlp(0)
    upsample_h(0, yp0)
    pools(1)
    yp1 = upsample_w(1)
    mlp(1)
    finals(0)
    upsample_h(1, yp1)
    finals(1)


def build_kernel():
    nc = bacc.Bacc("TRN2", target_bir_lowering=False, debug=False,
                   num_devices=N_CORES)
    dd = nc.dram_tensor("dd", [128, BL * G * DG], BF16,
                        kind="ExternalInput").ap()
    sh = nc.dram_tensor("sh", [BL, C, HS, WS], BF16,
                        kind="ExternalInput").ap()
    wpack = nc.dram_tensor("wpack", [128, 6 * C], BF16,
                           kind="ExternalInput").ap()
    bpack = nc.dram_tensor("bpack", [128, 4], F32,
                           kind="ExternalInput").ap()
    out = nc.dram_tensor("out", [BL, C, HS, WS], BF16,
                         kind="ExternalOutput").ap()

    with tile.TileContext(nc) as tc, ExitStack() as ctx:
        _emit(ctx, tc, dd, sh, wpack, bpack, out)
    nc.compile()
    return nc


_NC = None


def _get_nc():
    global _NC
    if _NC is None:
        _NC = build_kernel()
    return _NC


def prepare_in_maps(deeper, shallower, w1, b1, w2, b2):
    bf = ml_dtypes.bfloat16
    # w1t transposed with per-chunk mean folds: shallow rows (0:256) carry
    # the 1/(64*64) shallow-pool mean; deeper rows (256:512) carry 1/64
    # because dd holds d/16 and the deeper mean is sum(d)/1024.
    w1t = np.ascontiguousarray(w1.T).astype(np.float32)     # [512, 256]
    w1t[0:256] *= np.float32(1.0 / 4096.0)
    w1t[256:512] *= np.float32(1.0 / 64.0)
    w2t = np.ascontiguousarray(w2.T).astype(np.float32)     # [256, 256]
    wp = np.empty((128, 6 * C), np.float32)
    # blocks: 0=shallow g0, 1=shallow g1, 2=deeper g0, 3=deeper g1
    wp[:, 0 * C:1 * C] = w1t[0:128]
    wp[:, 1 * C:2 * C] = w1t[128:256]
    wp[:, 2 * C:3 * C] = w1t[256:384]
    wp[:, 3 * C:4 * C] = w1t[384:512]
    wp[:, 4 * C:5 * C] = w2t[0:128]
    wp[:, 5 * C:6 * C] = w2t[128:256]
    wp = wp.astype(bf)
    bp = np.empty((128, 4), np.float32)
    bp[:, 0:2] = b1.astype(np.float32).reshape(2, 128).T
    bp[:, 2:4] = b2.astype(np.float32).reshape(2, 128).T
    # deeper: /16 prescale (makes both 3a+b passes exact), guard cells.
    d16 = (deeper.astype(np.float32) * np.float32(1.0 / 16.0)).astype(bf)
    dg = np.pad(d16, ((0, 0), (0, 0), (1, 1), (1, 1)), mode='edge')
    # shallower: even/odd column planes
    shf = shallower.astype(np.float32).astype(bf)
    shp = np.empty_like(shf)
    shp[..., 0:WD] = shf[..., 0::2]
    shp[..., WD:WS] = shf[..., 1::2]
    in_maps = []
    for i in range(N_CORES):
        # dd pack: [BL, G, 128, HG*HG] -> [128, BL*G*HG*HG]
        dc = dg[i * BL:(i + 1) * BL].reshape(BL, G, 128, DG)
        ddp = np.ascontiguousarray(
            dc.transpose(2, 0, 1, 3).reshape(128, BL * G * DG))
        in_maps.append({
            "dd": ddp, "wpack": wp, "bpack": bp,
            "sh": np.ascontiguousarray(shp[i * BL:(i + 1) * BL]),
        })
    return in_maps


def gather(results):
    outp = np.concatenate(
        [results[i]["out"] for i in range(N_CORES)], axis=0
    ).astype(np.float32)
    o = np.empty_like(outp)
    o[..., 0::2] = outp[..., 0:WD]
    o[..., 1::2] = outp[..., WD:WS]
    return o


def kernel(deeper, shallower, w1, b1, w2, b2):
    nc = _get_nc()
    in_maps = prepare_in_maps(deeper, shallower, w1, b1, w2, b2)
    res = bass_utils.run_bass_kernel_spmd(nc, in_maps, list(range(N_CORES)))
    return gather(res.results)
